# revision 17
# baseline (speedup 1.0000x reference)
"""Trainium2 Bass kernel for a GAT block (GATConv + LN + FFN + LN).

Self-contained: builds per-core shards on the host, compiles one SPMD Bass
program, runs it on 8 NeuronCores via run_bass_kernel_spmd, and reassembles
the full [50000, 128] output.

Per-core scheme (core c of 8, nodes permuted own-first per core):
  Phase A: [h | a_src | a_dst] = x @ [W_perm | W@Asrc | W@Adst] for all 50176
           (padded) nodes; rows stored to core-local DRAM as 512B records
           [h(f,h-major) 128bf | a_src 8bf | a_dst 8bf | pad] for gathering;
           a_dst of own nodes also kept on-chip (adst_sb).
  Phase B: edges with dst owned by the core (incl self-loops), grouped by
           128-node dst block, split lo/hi on the 32K int16 gather-index
           limit, padded per (block, stream) to 128-edge granules with a
           shared max-over-cores profile so all cores run one program.
           Per 4096-edge chunk: one dma_gather brings h+a_src per edge.
           a_dst per edge via PE: one-hot St[d,e] (built from a PE
           outer-product dl broadcast + DVE is_equal) times adst_blk, plus
           identity-matmul accumulate of a_src -> logits in PSUM.
           p = exp(leaky_relu(logits)); msg = h * p (packed bf16, 2x DVE);
           scatter per granule: psum[block] += S^T @ [msg | p] with a second
           one-hot S[e,d].
  Phase C: g = agg/denom; u = LN(x + g); ff = relu(u@W1 + b1)@W2 + b2;
           z = LN(u + ff), batched over groups of 4 node blocks.
"""
import numpy as np
import ml_dtypes

N = 50000
NCORES = 8
OWN = 6272             # nodes per core (49 tiles of 128)
NP = OWN * NCORES      # padded node count
BLK = 128              # aggregation block == node tile
NBLK = OWN // BLK      # 49
GR = 128               # edges per granule
CHUNK = 4096           # edges per gather chunk (32 granules)
GPC = CHUNK // GR      # granules per chunk
OCT = 4                # granules per dl-broadcast group
LO_LIM = 1 << 15
H, F, D = 8, 16, 128
ROWW = 256             # h_d row width in bf16 elems (512 B)
PAD_DL = 200.0         # sentinel dst_in_block for pad edges
LN_EPS = 1e-5
GT = 7                 # node tiles per phase-A psum group (392 = 56*7)
XB = 14                # node tiles per x DMA
GB = 4                 # node blocks per phase-C group

bf16 = ml_dtypes.bfloat16


def _wrap16(idx):
    L = idx.shape[0]
    w = idx.reshape(L // 16, 16).T.astype(np.int16)
    return np.tile(w, (8, 1))                      # [128, L/16]


def _bfr(x):
    return np.ascontiguousarray(x, dtype=np.float32).astype(bf16)


def _build_host_data(inputs):
    x = np.asarray(inputs["x"], np.float32)
    W = np.asarray(inputs["W_gat"], np.float32)
    att_src = np.asarray(inputs["att_src"], np.float32)
    att_dst = np.asarray(inputs["att_dst"], np.float32)
    ei = np.asarray(inputs["edge_index"])

    src = ei[0].astype(np.int64)
    dst = ei[1].astype(np.int64)
    loops = np.arange(N, dtype=np.int64)
    src = np.concatenate([src, loops])
    dst = np.concatenate([dst, loops])

    # per-core own-first permutation; row index of global node n on core c:
    #   own nodes -> [0, OWN); others keep relative order after them
    perms = []      # perms[c][row] = global node
    invs = []       # invs[c][global padded node] = row
    allp = np.arange(NP, dtype=np.int64)
    for c in range(NCORES):
        own = allp[OWN * c: OWN * (c + 1)]
        rest = np.concatenate([allp[: OWN * c], allp[OWN * (c + 1):]])
        perm = np.concatenate([own, rest])
        inv = np.empty(NP, dtype=np.int64)
        inv[perm] = np.arange(NP)
        perms.append(perm)
        invs.append(inv)

    # per (core, block, stream) counts on permuted gather indices
    counts = np.zeros((NCORES, NBLK, 2), dtype=np.int64)
    core_edges = []
    for c in range(NCORES):
        m = (dst >= OWN * c) & (dst < min(OWN * (c + 1), N))
        s_g = invs[c][src[m]]                     # permuted gather row
        d_l = dst[m] - OWN * c                    # own-local dst == row (own-first)
        blk = d_l // BLK
        lo = s_g < LO_LIM
        core_edges.append((s_g, d_l, blk, lo))
        for b in range(NBLK):
            mb = blk == b
            counts[c, b, 0] = np.sum(mb & lo)
            counts[c, b, 1] = np.sum(mb & ~lo)

    g_prof = np.ceil(counts.max(axis=0) / GR).astype(np.int64)   # [NBLK, 2]
    L = [int(g_prof[:, s].sum()) * GR for s in range(2)]
    for s in range(2):
        pad = (-L[s]) % CHUNK
        g_prof[NBLK - 1, s] += pad // GR
        L[s] += pad
    L_LO, L_HI = L

    per_core = []
    for c in range(NCORES):
        s_g, d_l, blk, lo = core_edges[c]
        streams = []
        for sidx in range(2):
            mm = lo if sidx == 0 else ~lo
            Ls = L[sidx]
            gidx = np.zeros(Ls, dtype=np.int64)
            dl = np.full(Ls, PAD_DL, dtype=np.float32)
            pos = 0
            for b in range(NBLK):
                mb = (blk == b) & mm
                k = int(np.sum(mb))
                cap = int(g_prof[b, sidx]) * GR
                gidx[pos:pos + k] = s_g[mb] - (0 if sidx == 0 else LO_LIM)
                dl[pos:pos + k] = (d_l[mb] % BLK).astype(np.float32)
                pos += cap
            streams.append({
                "gidx16": _wrap16(gidx),
                # [128, L/128]: partition = edge-in-granule, free = granule
                "dlt": np.ascontiguousarray(dl.astype(bf16).reshape(-1, GR).T),
                # [1, L]: natural granule-major stream for PE outer-product
                "dlrow": np.ascontiguousarray(dl.astype(bf16).reshape(1, -1)),
            })
        per_core.append(streams)

    # block id of each granule per stream (chunk pad lands on last block)
    blk_of = []
    for s in range(2):
        bo = []
        for b in range(NBLK):
            bo += [b] * int(g_prof[b, s])
        blk_of.append(bo)

    # ---- weights ----
    # W_perm: col (f*8 + h) = W col (h*16 + f)  -> gathered h rows are
    # (f,h)-major so the msg multiply has a packed last dim (head).
    perm_fh = np.empty(D, dtype=np.int64)
    for f in range(F):
        for h in range(H):
            perm_fh[f * H + h] = h * F + f
    W_perm = W[:, perm_fh]
    Asrc = np.zeros((D, H), np.float32)
    Adst = np.zeros((D, H), np.float32)
    for h in range(H):
        Asrc[h * F:(h + 1) * F, h] = att_src[h]
        Adst[h * F:(h + 1) * F, h] = att_dst[h]
    Wext = _bfr(np.concatenate([W_perm, W @ Asrc, W @ Adst], axis=1))  # [128,144]

    iotaP = _bfr(np.tile(np.arange(128, dtype=np.float32).reshape(128, 1),
                         (1, 128)))                                   # val = p
    iotaRep = _bfr(np.tile(
        np.repeat(np.arange(BLK, dtype=np.float32), GPC).reshape(1, -1),
        (128, 1)))                                 # [128, BLK*GPC], val = d
    I128 = _bfr(np.eye(128, dtype=np.float32))

    xp = np.zeros((NP, D), np.float32)
    xp[:N] = x
    xT_per_core = []
    x_own_per_core = []
    for c in range(NCORES):
        xTc = np.ascontiguousarray(xp[perms[c]].T.astype(bf16))  # [128, NP]
        xT_per_core.append(xTc)
        x_own_per_core.append(np.ascontiguousarray(xp[OWN * c: OWN * (c + 1)]))

    host = {
        "g_prof": g_prof, "L_LO": L_LO, "L_HI": L_HI, "blk_of": blk_of,
        "per_core": per_core, "xT": xT_per_core, "x_own": x_own_per_core,
        "Wext": Wext, "iotaP": iotaP, "iotaRep": iotaRep, "I128": I128,
        "W1": _bfr(np.asarray(inputs["w_ff1"], np.float32)),     # [128,256]
        "W2": _bfr(np.asarray(inputs["w_ff2"], np.float32)),     # [256,128]
        "b1col": np.ascontiguousarray(
            np.asarray(inputs["b_ff1"], np.float32).reshape(2, 128).T),  # [128,2]
    }
    host["bias_gat"] = np.asarray(inputs["bias_gat"], np.float32)
    host["b_ff2"] = np.asarray(inputs["b_ff2"], np.float32)
    for nm in ("gamma1", "beta1", "gamma2", "beta2"):
        host[nm] = np.asarray(inputs[nm], np.float32)
    host["triv_gb1"] = bool(np.all(host["gamma1"] == 1) and np.all(host["beta1"] == 0))
    host["triv_gb2"] = bool(np.all(host["gamma2"] == 1) and np.all(host["beta2"] == 0))
    host["triv_bgat"] = bool(np.all(host["bias_gat"] == 0))
    host["triv_bff2"] = bool(np.all(host["b_ff2"] == 0))
    return host


def _build_program(host, phases="ABC"):
    import concourse.bacc as bacc
    import concourse.mybir as mybir
    import concourse.tile as tile
    from concourse.bass import AP

    fp32 = mybir.dt.float32
    bft = mybir.dt.bfloat16
    i16 = mybir.dt.int16
    Alu = mybir.AluOpType
    Act = mybir.ActivationFunctionType

    g_prof = host["g_prof"]
    L_LO, L_HI = host["L_LO"], host["L_HI"]
    blk_of = host["blk_of"]

    nc = bacc.Bacc("TRN2")

    # ---- DRAM tensors ----
    xT_d = nc.dram_tensor("xT", [128, NP], bft, kind="ExternalInput")
    xown_d = nc.dram_tensor("x_own", [OWN, D], fp32, kind="ExternalInput")
    Wext_d = nc.dram_tensor("Wext", [128, 144], bft, kind="ExternalInput")
    iotaP_d = nc.dram_tensor("iotaP", [128, 128], bft, kind="ExternalInput")
    iotaR_d = nc.dram_tensor("iotaRep", [128, BLK * GPC], bft, kind="ExternalInput")
    I128_d = nc.dram_tensor("I128", [128, 128], bft, kind="ExternalInput")
    W1_d = nc.dram_tensor("W1", [128, 256], bft, kind="ExternalInput")
    W2_d = nc.dram_tensor("W2", [256, 128], bft, kind="ExternalInput")
    b1c_d = nc.dram_tensor("b1col", [128, 2], fp32, kind="ExternalInput")
    gl_d = {}
    if not host["triv_bgat"]:
        gl_d["bgat"] = nc.dram_tensor("bgat_r", [128, 128], fp32, kind="ExternalInput")
    if not host["triv_bff2"]:
        gl_d["bff2"] = nc.dram_tensor("bff2_r", [128, 128], fp32, kind="ExternalInput")
    if not host["triv_gb1"]:
        gl_d["g1"] = nc.dram_tensor("g1_r", [128, 128], fp32, kind="ExternalInput")
        gl_d["b1"] = nc.dram_tensor("b1_r", [128, 128], fp32, kind="ExternalInput")
    if not host["triv_gb2"]:
        gl_d["g2"] = nc.dram_tensor("g2_r", [128, 128], fp32, kind="ExternalInput")
        gl_d["b2"] = nc.dram_tensor("b2_r", [128, 128], fp32, kind="ExternalInput")

    st_d = []
    for sname, Ls in (("lo", L_LO), ("hi", L_HI)):
        st_d.append({
            "gidx": nc.dram_tensor(f"gidx_{sname}", [128, Ls // 16], i16,
                                   kind="ExternalInput"),
            "dlt": nc.dram_tensor(f"dlt_{sname}", [128, Ls // GR], bft,
                                  kind="ExternalInput"),
            "dlrow": nc.dram_tensor(f"dlrow_{sname}", [1, Ls], bft,
                                    kind="ExternalInput"),
            "L": Ls,
        })

    h_d = nc.dram_tensor("h_scratch", [NP, ROWW], bft, kind="Internal")
    z_d = nc.dram_tensor("z", [OWN, D], fp32, kind="ExternalOutput")

    NT = NP // 128                    # 392 node tiles
    with tile.TileContext(nc) as tc:
        # ================= consts =================
        cpool = tc.alloc_tile_pool(name="consts", bufs=1)
        Wext_s = cpool.tile([128, 144], bft)
        nc.sync.dma_start(out=Wext_s[:], in_=Wext_d[:])
        iotaP_s = cpool.tile([128, 128], bft)
        nc.sync.dma_start(out=iotaP_s[:], in_=iotaP_d[:])
        iotaR_s = cpool.tile([128, BLK * GPC], bft)
        nc.sync.dma_start(out=iotaR_s[:], in_=iotaR_d[:])
        I128_s = cpool.tile([128, 128], bft)
        nc.sync.dma_start(out=I128_s[:], in_=I128_d[:])
        W1_s = cpool.tile([128, 256], bft)
        nc.sync.dma_start(out=W1_s[:], in_=W1_d[:])
        W2_s = cpool.tile([256 // 2, 2, 128], bft)   # [128, 2, 128]: chunk k rows
        nc.sync.dma_start(out=W2_s[:],
                          in_=W2_d[:].rearrange("(k h) f -> h k f", k=2))
        b1c_s = cpool.tile([128, 2], fp32)
        nc.sync.dma_start(out=b1c_s[:], in_=b1c_d[:])
        gl_s = {}
        for k, dref in gl_d.items():
            gl_s[k] = cpool.tile([128, 128], fp32, tag=f"gl_{k}")
            nc.sync.dma_start(out=gl_s[k][:], in_=dref[:])
        ones1 = cpool.tile([1, 128], bft)
        nc.vector.memset(ones1[:], 1.0)
        eps_s = cpool.tile([128, 1], fp32)
        nc.vector.memset(eps_s[:], LN_EPS)
        adst_sb = cpool.tile([128, NBLK * 8], bft)   # own-node a_dst per block

        # ================= phase A =================
        with tc.tile_pool(name="pA", bufs=3) as pA, \
             tc.tile_pool(name="psA", bufs=2, space="PSUM") as psA:
            xt = None
            for tg in range(NT // GT):
                t0 = tg * GT
                if t0 % XB == 0:
                    xt = pA.tile([128, XB * 128], bft, tag="xt")
                    nc.sync.dma_start(out=xt[:],
                                      in_=xT_d[:, t0 * 128:(t0 + XB) * 128])
                ps = psA.tile([128, GT, 144], fp32, tag="psA",
                              padded_shape=[128, GT, 256])
                for j in range(GT):
                    jo = (t0 % XB) + j
                    nc.tensor.matmul(ps[:, j, :],
                                     lhsT=xt[:, jo * 128:(jo + 1) * 128],
                                     rhs=Wext_s[:], start=True, stop=True)
                stage = pA.tile([128, GT, ROWW], bft, tag="stage")
                if tg < 3:  # first pool rotation: clear pad cols once
                    nc.gpsimd.memset(stage[:], 0.0)
                eng = nc.scalar if tg % 2 == 0 else nc.vector
                if eng is nc.scalar:
                    nc.scalar.activation(out=stage[:, :, 0:144], in_=ps[:],
                                         func=Act.Copy)
                else:
                    nc.vector.tensor_copy(out=stage[:, :, 0:144], in_=ps[:])
                if t0 < NBLK:  # own tiles: stash a_dst on-chip
                    ntl = min(GT, NBLK - t0)
                    nc.vector.tensor_copy(
                        out=adst_sb[:, t0 * 8:(t0 + ntl) * 8].rearrange(
                            "p (t e) -> p t e", e=8),
                        in_=ps[:, :ntl, 136:144])
                nc.sync.dma_start(
                    out=h_d[t0 * 128:(t0 + GT) * 128, :].rearrange(
                        "(j n) d -> n j d", j=GT),
                    in_=stage[:])

        tc.strict_bb_all_engine_barrier()

        # ================= phases B + C =================
        run_B = "B" in phases
        run_C = "C" in phases
        h_lo = h_d[0:LO_LIM, :]
        h_hi = h_d[LO_LIM:NP, :]
        starts = np.zeros((NBLK, 2), dtype=np.int64)   # granule start per block
        for s in range(2):
            starts[1:, s] = np.cumsum(g_prof[:-1, s])

        pB = tc.alloc_tile_pool(name="pB", bufs=2)
        pBs = tc.alloc_tile_pool(name="pBsmall", bufs=2)
        psDl = tc.alloc_tile_pool(name="psDl", bufs=1, space="PSUM")
        psAd = tc.alloc_tile_pool(name="psAd", bufs=1, space="PSUM")
        psB = tc.alloc_tile_pool(name="psB", bufs=2, space="PSUM")
        pC = tc.alloc_tile_pool(name="pC", bufs=2)
        psC = tc.alloc_tile_pool(name="psC", bufs=1, space="PSUM")

        chunk_tiles = [{}, {}]        # per stream: chunk idx -> tiles

        def emit_chunk(s, k):
            if k in chunk_tiles[s]:
                return chunk_tiles[s][k]
            sd = st_d[s]
            gix = pBs.tile([128, CHUNK // 16], i16, tag="gix")
            nc.sync.dma_start(out=gix[:],
                              in_=sd["gidx"][:, k * (CHUNK // 16):(k + 1) * (CHUNK // 16)])
            dlt = pBs.tile([128, GPC], bft, tag="dlt")
            nc.sync.dma_start(out=dlt[:],
                              in_=sd["dlt"][:, k * GPC:(k + 1) * GPC])
            dlr = pBs.tile([1, CHUNK], bft, tag="dlr")
            nc.sync.dma_start(out=dlr[:],
                              in_=sd["dlrow"][:, k * CHUNK:(k + 1) * CHUNK])
            h_ch = pB.tile([128, GPC, ROWW], bft, tag="h")
            nc.gpsimd.dma_gather(h_ch[:], h_lo if s == 0 else h_hi, gix[:],
                                 CHUNK, CHUNK, ROWW, single_packet=False)
            # St[d, g, e] one-hot: PE outer-product dl broadcast + is_equal
            St = pB.tile([128, GPC, 128], bft, tag="St")
            for o in range(GPC // OCT):
                dlb = psDl.tile([128, OCT, 128], fp32, tag="dlb")
                for gi in range(OCT):
                    g = o * OCT + gi
                    nc.tensor.matmul(dlb[:, gi, :], lhsT=ones1[:],
                                     rhs=dlr[0:1, g * GR:(g + 1) * GR],
                                     start=True, stop=True)
                ia = iotaP_s[:]
                i_b = AP(ia.tensor, ia.offset, [ia.ap[0], [0, OCT], [1, 128]])
                nc.vector.tensor_tensor(out=St[:, o * OCT:(o + 1) * OCT, :],
                                        in0=dlb[:], in1=i_b, op=Alu.is_equal)
            # S[e, d, g] one-hot for the scatter
            S = pB.tile([128, BLK, GPC], bft, tag=f"S{s}")
            da = dlt[:]
            dlt_b = AP(da.tensor, da.offset, [da.ap[0], [0, BLK], [1, GPC]])
            ra = iotaR_s[:]
            ir_b = AP(ra.tensor, ra.offset, [ra.ap[0], [GPC, BLK], [1, GPC]])
            nc.vector.tensor_tensor(out=S[:], in0=dlt_b, in1=ir_b,
                                    op=Alu.is_equal)
            # logits in PSUM: St^T @ adst_blk + I^T @ a_srcE
            adE = psAd.tile([128, GPC, 8], fp32, tag="adE")
            for g in range(GPC):
                b = blk_of[s][k * GPC + g]
                nc.tensor.matmul(adE[:, g, :], lhsT=St[:, g, :],
                                 rhs=adst_sb[:, b * 8:(b + 1) * 8],
                                 start=True, stop=False)
                nc.tensor.matmul(adE[:, g, :], lhsT=I128_s[:],
                                 rhs=h_ch[:, g, 128:136],
                                 start=False, stop=True)
            eLs = pBs.tile([128, GPC, 8], fp32, tag="eLs")
            nc.scalar.activation(out=eLs[:], in_=adE[:], func=Act.Copy)
            if "q" in phases and s == 0 and k == 0:
                nc.gpsimd.dma_start(out=z_d[0:128, :], in_=St[:, 0, :])
                nc.gpsimd.dma_start(out=z_d[128:256, :],
                                  in_=eLs[:, 0:16, :].rearrange("p g e -> p (g e)"))
                dbg2 = pBs.tile([128, 128], fp32, tag="dbq")
                nc.vector.tensor_copy(out=dbg2[:], in_=dlb[:, 0, :])
                nc.sync.dma_start(out=z_d[256:384, :], in_=dbg2[:])
            eL2 = pBs.tile([128, GPC, 8], bft, tag="eL")
            nc.vector.scalar_tensor_tensor(out=eL2[:], in0=eLs[:], scalar=0.2,
                                           in1=eLs[:], op0=Alu.mult, op1=Alu.max)
            msgp = pB.tile([128, GPC, 136], bft, tag=f"m{s}")
            nc.scalar.activation(out=msgp[:, :, 128:136], in_=eL2[:],
                                 func=Act.Exp)
            # msg = h * p, iterated (g, f, h) so every operand is packed bf16
            ma = msgp[:]
            ha = h_ch[:]
            out_ap = AP(ma.tensor, ma.offset,
                        [ma.ap[0], [136, GPC], [8, 16], [1, 8]])
            in0_ap = AP(ha.tensor, ha.offset,
                        [ha.ap[0], [ROWW, GPC], [8, 16], [1, 8]])
            in1_ap = AP(ma.tensor, ma.offset + 128,
                        [ma.ap[0], [136, GPC], [0, 16], [1, 8]])
            nc.vector.tensor_tensor(out=out_ap, in0=in0_ap, in1=in1_ap,
                                    op=Alu.mult)
            res = {"S": S, "msgp": msgp}
            chunk_tiles[s][k] = res
            return res

        gt_grp = None
        grp_b0 = 0
        for b in range(NBLK if run_B else 0):
            ps_blk = psB.tile([128, 136], fp32, tag="blk",
                              padded_shape=[128, 256])
            tot = int(g_prof[b, 0] + g_prof[b, 1])
            done = 0
            for s in range(2):
                for gi in range(int(g_prof[b, s])):
                    gg = int(starts[b, s]) + gi
                    ct = emit_chunk(s, gg // GPC)
                    gl = gg % GPC
                    Sa = ct["S"][:]
                    lhs = AP(Sa.tensor, Sa.offset + gl, [Sa.ap[0], [GPC, BLK]])
                    nc.tensor.matmul(ps_blk[:],
                                     lhsT=lhs,
                                     rhs=ct["msgp"][:, gl, :],
                                     start=(done == 0), stop=(done == tot - 1))
                    done += 1
            if "q" in phases:
                continue
            # normalize: gt[d, (h,f)] = ps[d, (f,h)] * (1/denom[d,h])
            if gt_grp is None:
                grp_b0 = b
                gt_grp = pC.tile([128, GB, 128], fp32, tag="gt")
            bi = b - grp_b0
            rec = pBs.tile([128, 8], fp32, tag="rec")
            nc.vector.reciprocal(out=rec[:], in_=ps_blk[:, 128:136])
            ga = gt_grp[:]
            pa = ps_blk[:]
            re = rec[:]
            gt_ap = AP(ga.tensor, ga.offset + bi * 128,
                       [ga.ap[0], [16, 8], [1, 16]])
            ps_ap = AP(pa.tensor, pa.offset, [pa.ap[0], [1, 8], [8, 16]])
            rc_ap = AP(re.tensor, re.offset, [re.ap[0], [1, 8], [0, 16]])
            nc.vector.tensor_tensor(out=gt_ap, in0=ps_ap, in1=rc_ap,
                                    op=Alu.mult)
            if not host["triv_bgat"]:
                bg = gl_s["bgat"][:]
                bg_b = AP(bg.tensor, bg.offset, [bg.ap[0], [0, 1], [1, 128]])
                nc.vector.tensor_tensor(out=gt_grp[:, bi:bi + 1, :],
                                        in0=gt_grp[:, bi:bi + 1, :],
                                        in1=bg_b, op=Alu.add)

            if "n" in phases or "d" in phases:
                dbg = pC.tile([128, 128], fp32, tag="dbg")
                nc.vector.memset(dbg[:], 0.0)
                if "n" in phases:
                    nc.vector.tensor_copy(out=dbg[:], in_=ps_blk[:, 0:128])
                else:
                    nc.vector.tensor_copy(out=dbg[:, 0:8], in_=ps_blk[:, 128:136])
                nc.sync.dma_start(out=z_d[b * 128:(b + 1) * 128, :], in_=dbg[:])
                gt_grp = None
                continue
            last_of_grp = (bi == GB - 1) or (b == NBLK - 1)
            if not last_of_grp:
                continue
            gb = bi + 1
            b0 = grp_b0
            gt_cur = gt_grp
            gt_grp = None
            if not run_C:
                nc.sync.dma_start(
                    out=z_d[b0 * 128:(b0 + gb) * 128, :].rearrange(
                        "(j n) d -> n j d", j=gb),
                    in_=gt_cur[:, :gb, :])
                continue
            # ---- phase C for blocks [b0, b0+gb) ----
            xo = pC.tile([128, GB, 128], fp32, tag="xo")
            nc.sync.dma_start(
                out=xo[:, :gb, :],
                in_=xown_d[b0 * 128:(b0 + gb) * 128, :].rearrange(
                    "(j n) d -> n j d", j=gb))
            t1 = pC.tile([128, GB, 128], fp32, tag="t1")
            nc.vector.tensor_tensor(out=t1[:, :gb, :], in0=xo[:, :gb, :],
                                    in1=gt_cur[:, :gb, :], op=Alu.add)

            def layer_norm(tin, g_key, b_key, triv, tagp):
                bst = pBs.tile([128, GB, 6], fp32, tag=f"bst{tagp}")
                mv = pBs.tile([128, GB, 2], fp32, tag=f"mv{tagp}")
                for i in range(gb):
                    nc.vector.bn_stats(out=bst[:, i, :], in_=tin[:, i, :])
                for i in range(gb):
                    nc.vector.bn_aggr(out=mv[:, i, :], in_=bst[:, i, :])
                # inv-std = exp(-0.5 * ln(var + eps)); Ln/Exp/Relu/Copy all
                # live in one Act table set -> no table reloads
                nc.scalar.activation(out=mv[:, :gb, 1:2], in_=mv[:, :gb, 1:2],
                                     func=Act.Ln, bias=eps_s[:])
                nc.scalar.activation(out=mv[:, :gb, 1:2], in_=mv[:, :gb, 1:2],
                                     func=Act.Exp, scale=-0.5)
                o = pC.tile([128, GB, 128], fp32, tag=f"ln{tagp}")
                for i in range(gb):
                    nc.vector.tensor_scalar(out=o[:, i, :], in0=tin[:, i, :],
                                            scalar1=mv[:, i, 0:1],
                                            op0=Alu.subtract,
                                            scalar2=mv[:, i, 1:2], op1=Alu.mult)
                if not triv:
                    for key, op in ((g_key, Alu.mult), (b_key, Alu.add)):
                        gv = gl_s[key][:]
                        g_b = AP(gv.tensor, gv.offset,
                                 [gv.ap[0], [0, gb], [1, 128]])
                        nc.vector.tensor_tensor(out=o[:, :gb, :],
                                                in0=o[:, :gb, :], in1=g_b,
                                                op=op)
                return o

            u = layer_norm(t1, "g1", "b1", host["triv_gb1"], "1")
            u_bf = pC.tile([128, GB, 128], bft, tag="ubf")
            nc.scalar.activation(out=u_bf[:, :gb, :], in_=u[:, :gb, :],
                                 func=Act.Copy)
            uT_ps = psC.tile([128, GB, 128], bft, tag="uT")
            for i in range(gb):
                nc.tensor.transpose(uT_ps[:, i, :], in_=u_bf[:, i, :],
                                    identity=I128_s[:])
            uTs = pC.tile([128, GB, 128], bft, tag="uTs")
            nc.scalar.activation(out=uTs[:, :gb, :], in_=uT_ps[:, :gb, :],
                                 func=Act.Copy)
            f1ps = psC.tile([128, 2, GB, 128], fp32, tag="f1")
            for j in range(2):
                nc.tensor.matmul(f1ps[:, j, :gb, :],
                                 lhsT=W1_s[:, j * 128:(j + 1) * 128],
                                 rhs=uTs[:, :gb, :], start=True, stop=True)
            r1 = pC.tile([128, 2, GB, 128], bft, tag="r1")
            for j in range(2):
                nc.scalar.activation(out=r1[:, j, :gb, :], in_=f1ps[:, j, :gb, :],
                                     func=Act.Relu, bias=b1c_s[:, j:j + 1])
            zps = psC.tile([128, GB, 128], fp32, tag="zp")
            for i in range(gb):
                for j in range(2):
                    nc.tensor.matmul(zps[:, i, :], lhsT=r1[:, j, i, :],
                                     rhs=W2_s[:, j, :],
                                     start=(j == 0), stop=(j == 1))
            t2 = pC.tile([128, GB, 128], fp32, tag="t2")
            nc.vector.tensor_tensor(out=t2[:, :gb, :], in0=u[:, :gb, :],
                                    in1=zps[:, :gb, :], op=Alu.add)
            if not host["triv_bff2"]:
                bf2 = gl_s["bff2"][:]
                b_b = AP(bf2.tensor, bf2.offset, [bf2.ap[0], [0, gb], [1, 128]])
                nc.vector.tensor_tensor(out=t2[:, :gb, :], in0=t2[:, :gb, :],
                                        in1=b_b, op=Alu.add)
            zt = layer_norm(t2, "g2", "b2", host["triv_gb2"], "2")
            nc.sync.dma_start(
                out=z_d[b0 * 128:(b0 + gb) * 128, :].rearrange(
                    "(j n) d -> n j d", j=gb),
                in_=zt[:, :gb, :])

        for p in (psC, pC, psB, psAd, psDl, pBs, pB):
            p.release()
        cpool.release()

    nc.compile()
    return nc


def kernel(**inputs):
    import os
    from concourse.bass_utils import run_bass_kernel_spmd

    host = _build_host_data(inputs)
    nc = _build_program(host, phases=os.environ.get("GAT_PHASES", "ABC"))

    in_maps = []
    for c in range(NCORES):
        m = {
            "xT": host["xT"][c],
            "x_own": host["x_own"][c],
            "Wext": host["Wext"], "iotaP": host["iotaP"],
            "iotaRep": host["iotaRep"], "I128": host["I128"],
            "W1": host["W1"], "W2": host["W2"], "b1col": host["b1col"],
        }
        if not host["triv_bgat"]:
            m["bgat_r"] = np.tile(host["bias_gat"].reshape(1, -1), (128, 1))
        if not host["triv_bff2"]:
            m["bff2_r"] = np.tile(host["b_ff2"].reshape(1, -1), (128, 1))
        if not host["triv_gb1"]:
            m["g1_r"] = np.tile(host["gamma1"].reshape(1, -1), (128, 1))
            m["b1_r"] = np.tile(host["beta1"].reshape(1, -1), (128, 1))
        if not host["triv_gb2"]:
            m["g2_r"] = np.tile(host["gamma2"].reshape(1, -1), (128, 1))
            m["b2_r"] = np.tile(host["beta2"].reshape(1, -1), (128, 1))
        for s, sname in ((0, "lo"), (1, "hi")):
            sd = host["per_core"][c][s]
            m[f"gidx_{sname}"] = sd["gidx16"]
            m[f"dlt_{sname}"] = sd["dlt"]
            m[f"dlrow_{sname}"] = sd["dlrow"]
        in_maps.append(m)

    trace = bool(int(os.environ.get("GAT_TRACE", "0")))
    res = run_bass_kernel_spmd(nc, in_maps, core_ids=list(range(NCORES)),
                               trace=trace)
    if trace and res.exec_time_ns:
        print(f"HW exec time: {res.exec_time_ns} ns")
    if bool(int(os.environ.get("GAT_TIME", "0"))):
        try:
            from concourse.timeline_sim import TimelineSim
            ts = TimelineSim(nc)
            dur = ts.simulate()
            print(f"HW exec time: {dur:.0f} ns (cost-model timeline estimate)")
        except Exception as e:
            print("timeline sim failed:", e)

    out = np.zeros((N, D), np.float32)
    for c in range(NCORES):
        lo_n = OWN * c
        hi_n = min(OWN * (c + 1), N)
        out[lo_n:hi_n] = res.results[c]["z"][: hi_n - lo_n]
    return out


# revision 20
# speedup vs baseline: 1.1437x; 1.1437x over previous
"""Trainium2 Bass kernel for a GAT block (GATConv + LN + FFN + LN).

Self-contained: builds per-core shards on the host, compiles one SPMD Bass
program, runs it on 8 NeuronCores via run_bass_kernel_spmd, and reassembles
the full [50000, 128] output.

Per-core scheme (core c of 8, nodes permuted own-first per core):
  Phase A: [h | a_src | a_dst] = x @ [W_perm | W@Asrc | W@Adst] for all 50176
           (padded) nodes; rows stored to core-local DRAM as 512B records
           [h(f,h-major) 128bf | a_src 8bf | a_dst 8bf | pad] for gathering;
           a_dst of own nodes also kept on-chip (adst_sb).
  Phase B: edges with dst owned by the core (incl self-loops), grouped by
           128-node dst block, split lo/hi on the 32K int16 gather-index
           limit, padded per (block, stream) to 128-edge granules with a
           shared max-over-cores profile so all cores run one program.
           Per 4096-edge chunk: one dma_gather brings h+a_src per edge.
           a_dst per edge via PE: one-hot St[d,e] (built from a PE
           outer-product dl broadcast + DVE is_equal) times adst_blk, plus
           identity-matmul accumulate of a_src -> logits in PSUM.
           p = exp(leaky_relu(logits)); msg = h * p (packed bf16, 2x DVE);
           scatter per granule: psum[block] += S^T @ [msg | p] with a second
           one-hot S[e,d].
  Phase C: g = agg/denom; u = LN(x + g); ff = relu(u@W1 + b1)@W2 + b2;
           z = LN(u + ff), batched over groups of 4 node blocks.
"""
import numpy as np
import ml_dtypes

N = 50000
NCORES = 8
OWN = 6272             # nodes per core (49 tiles of 128)
NP = OWN * NCORES      # padded node count
BLK = 128              # aggregation block == node tile
NBLK = OWN // BLK      # 49
GR = 128               # edges per granule
CHUNK = 4096           # edges per gather chunk (32 granules)
GPC = CHUNK // GR      # granules per chunk
OCT = 8                # granules per dl-broadcast psum tile
LO_LIM = 1 << 15
H, F, D = 8, 16, 128
ROWW = 256             # h_d row width in bf16 elems (512 B)
PAD_DL = 200.0         # sentinel dst_in_block for pad edges
LN_EPS = 1e-5
GT = 7                 # node tiles per phase-A psum group (392 = 56*7)
XB = 14                # node tiles per x DMA
GB = 4                 # node blocks per phase-C group

bf16 = ml_dtypes.bfloat16


def _wrap16(idx):
    L = idx.shape[0]
    w = idx.reshape(L // 16, 16).T.astype(np.int16)
    return np.tile(w, (8, 1))                      # [128, L/16]


def _bfr(x):
    return np.ascontiguousarray(x, dtype=np.float32).astype(bf16)


def _build_host_data(inputs):
    x = np.asarray(inputs["x"], np.float32)
    W = np.asarray(inputs["W_gat"], np.float32)
    att_src = np.asarray(inputs["att_src"], np.float32)
    att_dst = np.asarray(inputs["att_dst"], np.float32)
    ei = np.asarray(inputs["edge_index"])

    src = ei[0].astype(np.int64)
    dst = ei[1].astype(np.int64)
    loops = np.arange(N, dtype=np.int64)
    src = np.concatenate([src, loops])
    dst = np.concatenate([dst, loops])

    # per-core own-first permutation; row index of global node n on core c:
    #   own nodes -> [0, OWN); others keep relative order after them
    perms = []      # perms[c][row] = global node
    invs = []       # invs[c][global padded node] = row
    allp = np.arange(NP, dtype=np.int64)
    for c in range(NCORES):
        own = allp[OWN * c: OWN * (c + 1)]
        rest = np.concatenate([allp[: OWN * c], allp[OWN * (c + 1):]])
        perm = np.concatenate([own, rest])
        inv = np.empty(NP, dtype=np.int64)
        inv[perm] = np.arange(NP)
        perms.append(perm)
        invs.append(inv)

    # per (core, block, stream) counts on permuted gather indices
    counts = np.zeros((NCORES, NBLK, 2), dtype=np.int64)
    core_edges = []
    for c in range(NCORES):
        m = (dst >= OWN * c) & (dst < min(OWN * (c + 1), N))
        s_g = invs[c][src[m]]                     # permuted gather row
        d_l = dst[m] - OWN * c                    # own-local dst == row (own-first)
        blk = d_l // BLK
        lo = s_g < LO_LIM
        core_edges.append((s_g, d_l, blk, lo))
        for b in range(NBLK):
            mb = blk == b
            counts[c, b, 0] = np.sum(mb & lo)
            counts[c, b, 1] = np.sum(mb & ~lo)

    g_prof = np.ceil(counts.max(axis=0) / GR).astype(np.int64)   # [NBLK, 2]
    L = [int(g_prof[:, s].sum()) * GR for s in range(2)]
    for s in range(2):
        pad = (-L[s]) % CHUNK
        g_prof[NBLK - 1, s] += pad // GR
        L[s] += pad
    L_LO, L_HI = L

    per_core = []
    for c in range(NCORES):
        s_g, d_l, blk, lo = core_edges[c]
        streams = []
        for sidx in range(2):
            mm = lo if sidx == 0 else ~lo
            Ls = L[sidx]
            gidx = np.zeros(Ls, dtype=np.int64)
            dl = np.full(Ls, PAD_DL, dtype=np.float32)
            pos = 0
            for b in range(NBLK):
                mb = (blk == b) & mm
                k = int(np.sum(mb))
                cap = int(g_prof[b, sidx]) * GR
                gidx[pos:pos + k] = s_g[mb] - (0 if sidx == 0 else LO_LIM)
                dl[pos:pos + k] = (d_l[mb] % BLK).astype(np.float32)
                pos += cap
            streams.append({
                "gidx16": _wrap16(gidx),
                # [128, L/128]: partition = edge-in-granule, free = granule
                "dlt": np.ascontiguousarray(dl.astype(bf16).reshape(-1, GR).T),
            })
        per_core.append(streams)

    # block id of each granule per stream (chunk pad lands on last block)
    blk_of = []
    for s in range(2):
        bo = []
        for b in range(NBLK):
            bo += [b] * int(g_prof[b, s])
        blk_of.append(bo)

    # ---- weights ----
    # W_perm: col (f*8 + h) = W col (h*16 + f)  -> gathered h rows are
    # (f,h)-major so the msg multiply has a packed last dim (head).
    perm_fh = np.empty(D, dtype=np.int64)
    for f in range(F):
        for h in range(H):
            perm_fh[f * H + h] = h * F + f
    W_perm = W[:, perm_fh]
    Asrc = np.zeros((D, H), np.float32)
    Adst = np.zeros((D, H), np.float32)
    for h in range(H):
        Asrc[h * F:(h + 1) * F, h] = att_src[h]
        Adst[h * F:(h + 1) * F, h] = att_dst[h]
    Wext = _bfr(np.concatenate([W_perm, W @ Asrc, W @ Adst], axis=1))  # [128,144]

    iotaP = _bfr(np.tile(np.arange(128, dtype=np.float32).reshape(128, 1),
                         (1, 128)))                                   # val = p
    iotaRep = _bfr(np.tile(
        np.repeat(np.arange(BLK, dtype=np.float32), GPC).reshape(1, -1),
        (128, 1)))                                 # [128, BLK*GPC], val = d
    I128 = _bfr(np.eye(128, dtype=np.float32))

    xp = np.zeros((NP, D), np.float32)
    xp[:N] = x
    xT_per_core = []
    x_own_per_core = []
    for c in range(NCORES):
        xTc = np.ascontiguousarray(xp[perms[c]].T.astype(bf16))  # [128, NP]
        xT_per_core.append(xTc)
        x_own_per_core.append(np.ascontiguousarray(xp[OWN * c: OWN * (c + 1)]))

    host = {
        "g_prof": g_prof, "L_LO": L_LO, "L_HI": L_HI, "blk_of": blk_of,
        "per_core": per_core, "xT": xT_per_core, "x_own": x_own_per_core,
        "Wext": Wext, "iotaP": iotaP, "iotaRep": iotaRep, "I128": I128,
        "W1": _bfr(np.asarray(inputs["w_ff1"], np.float32)),     # [128,256]
        "W2": _bfr(np.asarray(inputs["w_ff2"], np.float32)),     # [256,128]
        "b1col": np.ascontiguousarray(
            np.asarray(inputs["b_ff1"], np.float32).reshape(2, 128).T),  # [128,2]
    }
    host["bias_gat"] = np.asarray(inputs["bias_gat"], np.float32)
    host["b_ff2"] = np.asarray(inputs["b_ff2"], np.float32)
    for nm in ("gamma1", "beta1", "gamma2", "beta2"):
        host[nm] = np.asarray(inputs[nm], np.float32)
    host["triv_gb1"] = bool(np.all(host["gamma1"] == 1) and np.all(host["beta1"] == 0))
    host["triv_gb2"] = bool(np.all(host["gamma2"] == 1) and np.all(host["beta2"] == 0))
    host["triv_bgat"] = bool(np.all(host["bias_gat"] == 0))
    host["triv_bff2"] = bool(np.all(host["b_ff2"] == 0))
    return host


def _build_program(host, phases="ABC"):
    import concourse.bacc as bacc
    import concourse.mybir as mybir
    import concourse.tile as tile
    from concourse.bass import AP

    fp32 = mybir.dt.float32
    bft = mybir.dt.bfloat16
    i16 = mybir.dt.int16
    Alu = mybir.AluOpType
    Act = mybir.ActivationFunctionType

    g_prof = host["g_prof"]
    L_LO, L_HI = host["L_LO"], host["L_HI"]
    blk_of = host["blk_of"]

    nc = bacc.Bacc("TRN2")

    # ---- DRAM tensors ----
    xT_d = nc.dram_tensor("xT", [128, NP], bft, kind="ExternalInput")
    xown_d = nc.dram_tensor("x_own", [OWN, D], fp32, kind="ExternalInput")
    Wext_d = nc.dram_tensor("Wext", [128, 144], bft, kind="ExternalInput")
    iotaP_d = nc.dram_tensor("iotaP", [128, 128], bft, kind="ExternalInput")
    iotaR_d = nc.dram_tensor("iotaRep", [128, BLK * GPC], bft, kind="ExternalInput")
    I128_d = nc.dram_tensor("I128", [128, 128], bft, kind="ExternalInput")
    W1_d = nc.dram_tensor("W1", [128, 256], bft, kind="ExternalInput")
    W2_d = nc.dram_tensor("W2", [256, 128], bft, kind="ExternalInput")
    b1c_d = nc.dram_tensor("b1col", [128, 2], fp32, kind="ExternalInput")
    gl_d = {}
    if not host["triv_bgat"]:
        gl_d["bgat"] = nc.dram_tensor("bgat_r", [128, 128], fp32, kind="ExternalInput")
    if not host["triv_bff2"]:
        gl_d["bff2"] = nc.dram_tensor("bff2_r", [128, 128], fp32, kind="ExternalInput")
    if not host["triv_gb1"]:
        gl_d["g1"] = nc.dram_tensor("g1_r", [128, 128], fp32, kind="ExternalInput")
        gl_d["b1"] = nc.dram_tensor("b1_r", [128, 128], fp32, kind="ExternalInput")
    if not host["triv_gb2"]:
        gl_d["g2"] = nc.dram_tensor("g2_r", [128, 128], fp32, kind="ExternalInput")
        gl_d["b2"] = nc.dram_tensor("b2_r", [128, 128], fp32, kind="ExternalInput")

    st_d = []
    for sname, Ls in (("lo", L_LO), ("hi", L_HI)):
        st_d.append({
            "gidx": nc.dram_tensor(f"gidx_{sname}", [128, Ls // 16], i16,
                                   kind="ExternalInput"),
            "dlt": nc.dram_tensor(f"dlt_{sname}", [128, Ls // GR], bft,
                                  kind="ExternalInput"),
            "L": Ls,
        })

    h_d = nc.dram_tensor("h_scratch", [NP, ROWW], bft, kind="Internal")
    z_d = nc.dram_tensor("z", [OWN, D], fp32, kind="ExternalOutput")

    NT = NP // 128                    # 392 node tiles
    with tile.TileContext(nc) as tc:
        # ================= consts =================
        cpool = tc.alloc_tile_pool(name="consts", bufs=1)
        Wext_s = cpool.tile([128, 144], bft)
        nc.sync.dma_start(out=Wext_s[:], in_=Wext_d[:])
        iotaP_s = cpool.tile([128, 128], bft)
        nc.sync.dma_start(out=iotaP_s[:], in_=iotaP_d[:])
        iotaR_s = cpool.tile([128, BLK * GPC], bft)
        nc.sync.dma_start(out=iotaR_s[:], in_=iotaR_d[:])
        I128_s = cpool.tile([128, 128], bft)
        nc.sync.dma_start(out=I128_s[:], in_=I128_d[:])
        W1_s = cpool.tile([128, 256], bft)
        nc.sync.dma_start(out=W1_s[:], in_=W1_d[:])
        W2_s = cpool.tile([256 // 2, 2, 128], bft)   # [128, 2, 128]: chunk k rows
        nc.sync.dma_start(out=W2_s[:],
                          in_=W2_d[:].rearrange("(k h) f -> h k f", k=2))
        b1c_s = cpool.tile([128, 2], fp32)
        nc.sync.dma_start(out=b1c_s[:], in_=b1c_d[:])
        gl_s = {}
        for k, dref in gl_d.items():
            gl_s[k] = cpool.tile([128, 128], fp32, tag=f"gl_{k}")
            nc.sync.dma_start(out=gl_s[k][:], in_=dref[:])
        eps_s = cpool.tile([128, 1], fp32)
        nc.vector.memset(eps_s[:], LN_EPS)
        adst_sb = cpool.tile([128, NBLK * 8], bft)   # own-node a_dst per block

        # ================= phase A =================
        with tc.tile_pool(name="pA", bufs=3) as pA, \
             tc.tile_pool(name="psA", bufs=2, space="PSUM") as psA:
            xt = None
            for tg in range(NT // GT):
                t0 = tg * GT
                if t0 % XB == 0:
                    xt = pA.tile([128, XB * 128], bft, tag="xt")
                    nc.sync.dma_start(out=xt[:],
                                      in_=xT_d[:, t0 * 128:(t0 + XB) * 128])
                ps = psA.tile([128, GT, 144], fp32, tag="psA",
                              padded_shape=[128, GT, 256])
                for j in range(GT):
                    jo = (t0 % XB) + j
                    nc.tensor.matmul(ps[:, j, :],
                                     lhsT=xt[:, jo * 128:(jo + 1) * 128],
                                     rhs=Wext_s[:], start=True, stop=True)
                stage = pA.tile([128, GT, ROWW], bft, tag="stage")
                if tg < 3:  # first pool rotation: clear pad cols once
                    nc.gpsimd.memset(stage[:], 0.0)
                nc.scalar.activation(out=stage[:, :, 0:144], in_=ps[:],
                                     func=Act.Copy)
                if t0 < NBLK:  # own tiles: stash a_dst on-chip
                    ntl = min(GT, NBLK - t0)
                    nc.vector.tensor_copy(
                        out=adst_sb[:, t0 * 8:(t0 + ntl) * 8].rearrange(
                            "p (t e) -> p t e", e=8),
                        in_=ps[:, :ntl, 136:144])
                nc.scalar.dma_start(
                    out=h_d[t0 * 128:(t0 + GT) * 128, :].rearrange(
                        "(j n) d -> n j d", j=GT),
                    in_=stage[:])

        tc.strict_bb_all_engine_barrier()

        # ================= phases B + C =================
        run_B = "B" in phases
        run_C = "C" in phases
        h_lo = h_d[0:LO_LIM, :]
        h_hi = h_d[LO_LIM:NP, :]
        starts = np.zeros((NBLK, 2), dtype=np.int64)   # granule start per block
        for s in range(2):
            starts[1:, s] = np.cumsum(g_prof[:-1, s])

        pB = tc.alloc_tile_pool(name="pB", bufs=2)
        pBs = tc.alloc_tile_pool(name="pBsmall", bufs=2)
        psDl = tc.alloc_tile_pool(name="psDl", bufs=1, space="PSUM")
        psAd = tc.alloc_tile_pool(name="psAd", bufs=1, space="PSUM")
        psB = tc.alloc_tile_pool(name="psB", bufs=2, space="PSUM")
        pC = tc.alloc_tile_pool(name="pC", bufs=2)
        psC = tc.alloc_tile_pool(name="psC", bufs=1, space="PSUM")

        chunk_tiles = [{}, {}]        # per stream: chunk idx -> tiles

        def emit_chunk(s, k):
            if k in chunk_tiles[s]:
                return chunk_tiles[s][k]
            sd = st_d[s]
            gix = pBs.tile([128, CHUNK // 16], i16, tag="gix")
            nc.sync.dma_start(out=gix[:],
                              in_=sd["gidx"][:, k * (CHUNK // 16):(k + 1) * (CHUNK // 16)])
            dlt = pBs.tile([128, GPC], bft, tag="dlt")
            nc.sync.dma_start(out=dlt[:],
                              in_=sd["dlt"][:, k * GPC:(k + 1) * GPC])
            h_ch = pB.tile([128, GPC, ROWW], bft, tag="h")
            nc.gpsimd.dma_gather(h_ch[:], h_lo if s == 0 else h_hi, gix[:],
                                 CHUNK, CHUNK, ROWW, single_packet=False)
            # St[d, g, e] one-hot: dl broadcast via transpose-matmul (bf16
            # PSUM keeps the is_equal in 2x mode), ping-ponging tile halves
            St = pB.tile([128, GPC, 128], bft, tag="St")
            dlb = psDl.tile([128, OCT, 128], bft, tag="dlb")
            da = dlt[:]
            HO = OCT // 2
            for g in range(GPC):
                lhs = AP(da.tensor, da.offset + g, [da.ap[0], [0, 128]])
                nc.tensor.matmul(dlb[:, g % OCT, :], lhsT=lhs, rhs=I128_s[:],
                                 start=True, stop=True, is_transpose=True)
                if g % HO == HO - 1:
                    ia = iotaP_s[:]
                    i_b = AP(ia.tensor, ia.offset,
                             [ia.ap[0], [0, HO], [1, 128]])
                    nc.vector.tensor_tensor(
                        out=St[:, g - HO + 1:g + 1, :],
                        in0=dlb[:, (g % OCT) - HO + 1:(g % OCT) + 1, :],
                        in1=i_b, op=Alu.is_equal)
            # S[e, d, g] one-hot for the scatter
            S = pB.tile([128, BLK, GPC], bft, tag=f"S{s}")
            dlt_b = AP(da.tensor, da.offset, [da.ap[0], [0, BLK], [1, GPC]])
            ra = iotaR_s[:]
            ir_b = AP(ra.tensor, ra.offset, [ra.ap[0], [GPC, BLK], [1, GPC]])
            nc.vector.tensor_tensor(out=S[:], in0=dlt_b, in1=ir_b,
                                    op=Alu.is_equal)
            # logits in PSUM: St^T @ adst_blk + I^T @ a_srcE
            adE = psAd.tile([128, GPC, 8], fp32, tag="adE")
            for g in range(GPC):
                b = blk_of[s][k * GPC + g]
                nc.tensor.matmul(adE[:, g, :], lhsT=St[:, g, :],
                                 rhs=adst_sb[:, b * 8:(b + 1) * 8],
                                 start=True, stop=False)
                nc.tensor.matmul(adE[:, g, :], lhsT=I128_s[:],
                                 rhs=h_ch[:, g, 128:136],
                                 start=False, stop=True)
            eLs = pBs.tile([128, GPC, 8], fp32, tag="eLs")
            nc.scalar.activation(out=eLs[:], in_=adE[:], func=Act.Copy)
            if "q" in phases and s == 0 and k == 0:
                nc.gpsimd.dma_start(out=z_d[0:128, :], in_=St[:, 0, :])
                nc.gpsimd.dma_start(out=z_d[128:256, :],
                                  in_=eLs[:, 0:16, :].rearrange("p g e -> p (g e)"))
                dbg2 = pBs.tile([128, 128], fp32, tag="dbq")
                nc.vector.tensor_copy(out=dbg2[:], in_=dlb[:, 0, :])
                nc.sync.dma_start(out=z_d[256:384, :], in_=dbg2[:])
            eL2 = pBs.tile([128, GPC, 8], bft, tag="eL")
            nc.vector.scalar_tensor_tensor(out=eL2[:], in0=eLs[:], scalar=0.2,
                                           in1=eLs[:], op0=Alu.mult, op1=Alu.max)
            msgp = pB.tile([128, GPC, 136], bft, tag=f"m{s}")
            nc.scalar.activation(out=msgp[:, :, 128:136], in_=eL2[:],
                                 func=Act.Exp)
            # msg = h * p, iterated (g, f, h) so every operand is packed bf16
            ma = msgp[:]
            ha = h_ch[:]
            out_ap = AP(ma.tensor, ma.offset,
                        [ma.ap[0], [136, GPC], [8, 16], [1, 8]])
            in0_ap = AP(ha.tensor, ha.offset,
                        [ha.ap[0], [ROWW, GPC], [8, 16], [1, 8]])
            in1_ap = AP(ma.tensor, ma.offset + 128,
                        [ma.ap[0], [136, GPC], [0, 16], [1, 8]])
            nc.vector.tensor_tensor(out=out_ap, in0=in0_ap, in1=in1_ap,
                                    op=Alu.mult)
            res = {"S": S, "msgp": msgp}
            chunk_tiles[s][k] = res
            return res

        gt_grp = None
        grp_b0 = 0
        for b in range(NBLK if run_B else 0):
            ps_blk = psB.tile([128, 136], fp32, tag="blk",
                              padded_shape=[128, 256])
            tot = int(g_prof[b, 0] + g_prof[b, 1])
            done = 0
            for s in range(2):
                for gi in range(int(g_prof[b, s])):
                    gg = int(starts[b, s]) + gi
                    ct = emit_chunk(s, gg // GPC)
                    gl = gg % GPC
                    Sa = ct["S"][:]
                    lhs = AP(Sa.tensor, Sa.offset + gl, [Sa.ap[0], [GPC, BLK]])
                    nc.tensor.matmul(ps_blk[:],
                                     lhsT=lhs,
                                     rhs=ct["msgp"][:, gl, :],
                                     start=(done == 0), stop=(done == tot - 1))
                    done += 1
            if "q" in phases:
                continue
            # normalize: gt[d, (h,f)] = ps[d, (f,h)] * (1/denom[d,h])
            if gt_grp is None:
                grp_b0 = b
                gt_grp = pC.tile([128, GB, 128], fp32, tag="gt")
            bi = b - grp_b0
            rec = pBs.tile([128, 8], fp32, tag="rec")
            nc.vector.reciprocal(out=rec[:], in_=ps_blk[:, 128:136])
            ga = gt_grp[:]
            pa = ps_blk[:]
            re = rec[:]
            gt_ap = AP(ga.tensor, ga.offset + bi * 128,
                       [ga.ap[0], [16, 8], [1, 16]])
            ps_ap = AP(pa.tensor, pa.offset, [pa.ap[0], [1, 8], [8, 16]])
            rc_ap = AP(re.tensor, re.offset, [re.ap[0], [1, 8], [0, 16]])
            nc.vector.tensor_tensor(out=gt_ap, in0=ps_ap, in1=rc_ap,
                                    op=Alu.mult)
            if not host["triv_bgat"]:
                bg = gl_s["bgat"][:]
                bg_b = AP(bg.tensor, bg.offset, [bg.ap[0], [0, 1], [1, 128]])
                nc.vector.tensor_tensor(out=gt_grp[:, bi:bi + 1, :],
                                        in0=gt_grp[:, bi:bi + 1, :],
                                        in1=bg_b, op=Alu.add)

            if "n" in phases or "d" in phases:
                dbg = pC.tile([128, 128], fp32, tag="dbg")
                nc.vector.memset(dbg[:], 0.0)
                if "n" in phases:
                    nc.vector.tensor_copy(out=dbg[:], in_=ps_blk[:, 0:128])
                else:
                    nc.vector.tensor_copy(out=dbg[:, 0:8], in_=ps_blk[:, 128:136])
                nc.sync.dma_start(out=z_d[b * 128:(b + 1) * 128, :], in_=dbg[:])
                gt_grp = None
                continue
            last_of_grp = (bi == GB - 1) or (b == NBLK - 1)
            if not last_of_grp:
                continue
            gb = bi + 1
            b0 = grp_b0
            gt_cur = gt_grp
            gt_grp = None
            if not run_C:
                nc.sync.dma_start(
                    out=z_d[b0 * 128:(b0 + gb) * 128, :].rearrange(
                        "(j n) d -> n j d", j=gb),
                    in_=gt_cur[:, :gb, :])
                continue
            # ---- phase C for blocks [b0, b0+gb) ----
            xo = pC.tile([128, GB, 128], fp32, tag="xo")
            nc.sync.dma_start(
                out=xo[:, :gb, :],
                in_=xown_d[b0 * 128:(b0 + gb) * 128, :].rearrange(
                    "(j n) d -> n j d", j=gb))
            t1 = pC.tile([128, GB, 128], fp32, tag="t1")
            nc.vector.tensor_tensor(out=t1[:, :gb, :], in0=xo[:, :gb, :],
                                    in1=gt_cur[:, :gb, :], op=Alu.add)

            def layer_norm(tin, g_key, b_key, triv, tagp):
                bst = pBs.tile([128, GB, 6], fp32, tag=f"bst{tagp}")
                mv = pBs.tile([128, GB, 2], fp32, tag=f"mv{tagp}")
                for i in range(gb):
                    nc.vector.bn_stats(out=bst[:, i, :], in_=tin[:, i, :])
                for i in range(gb):
                    nc.vector.bn_aggr(out=mv[:, i, :], in_=bst[:, i, :])
                # inv-std = exp(-0.5 * ln(var + eps)); Ln/Exp/Relu/Copy all
                # live in one Act table set -> no table reloads
                nc.scalar.activation(out=mv[:, :gb, 1:2], in_=mv[:, :gb, 1:2],
                                     func=Act.Ln, bias=eps_s[:])
                nc.scalar.activation(out=mv[:, :gb, 1:2], in_=mv[:, :gb, 1:2],
                                     func=Act.Exp, scale=-0.5)
                o = pC.tile([128, GB, 128], fp32, tag=f"ln{tagp}")
                for i in range(gb):
                    nc.vector.tensor_scalar(out=o[:, i, :], in0=tin[:, i, :],
                                            scalar1=mv[:, i, 0:1],
                                            op0=Alu.subtract,
                                            scalar2=mv[:, i, 1:2], op1=Alu.mult)
                if not triv:
                    for key, op in ((g_key, Alu.mult), (b_key, Alu.add)):
                        gv = gl_s[key][:]
                        g_b = AP(gv.tensor, gv.offset,
                                 [gv.ap[0], [0, gb], [1, 128]])
                        nc.vector.tensor_tensor(out=o[:, :gb, :],
                                                in0=o[:, :gb, :], in1=g_b,
                                                op=op)
                return o

            u = layer_norm(t1, "g1", "b1", host["triv_gb1"], "1")
            u_bf = pC.tile([128, GB, 128], bft, tag="ubf")
            nc.scalar.activation(out=u_bf[:, :gb, :], in_=u[:, :gb, :],
                                 func=Act.Copy)
            uT_ps = psC.tile([128, GB, 128], bft, tag="uT")
            for i in range(gb):
                nc.tensor.transpose(uT_ps[:, i, :], in_=u_bf[:, i, :],
                                    identity=I128_s[:])
            uTs = pC.tile([128, GB, 128], bft, tag="uTs")
            nc.scalar.activation(out=uTs[:, :gb, :], in_=uT_ps[:, :gb, :],
                                 func=Act.Copy)
            f1ps = psC.tile([128, 2, GB, 128], fp32, tag="f1")
            for j in range(2):
                nc.tensor.matmul(f1ps[:, j, :gb, :],
                                 lhsT=W1_s[:, j * 128:(j + 1) * 128],
                                 rhs=uTs[:, :gb, :], start=True, stop=True)
            r1 = pC.tile([128, 2, GB, 128], bft, tag="r1")
            for j in range(2):
                nc.scalar.activation(out=r1[:, j, :gb, :], in_=f1ps[:, j, :gb, :],
                                     func=Act.Relu, bias=b1c_s[:, j:j + 1])
            zps = psC.tile([128, GB, 128], fp32, tag="zp")
            for i in range(gb):
                for j in range(2):
                    nc.tensor.matmul(zps[:, i, :], lhsT=r1[:, j, i, :],
                                     rhs=W2_s[:, j, :],
                                     start=(j == 0), stop=(j == 1))
            t2 = pC.tile([128, GB, 128], fp32, tag="t2")
            nc.vector.tensor_tensor(out=t2[:, :gb, :], in0=u[:, :gb, :],
                                    in1=zps[:, :gb, :], op=Alu.add)
            if not host["triv_bff2"]:
                bf2 = gl_s["bff2"][:]
                b_b = AP(bf2.tensor, bf2.offset, [bf2.ap[0], [0, gb], [1, 128]])
                nc.vector.tensor_tensor(out=t2[:, :gb, :], in0=t2[:, :gb, :],
                                        in1=b_b, op=Alu.add)
            zt = layer_norm(t2, "g2", "b2", host["triv_gb2"], "2")
            nc.gpsimd.dma_start(
                out=z_d[b0 * 128:(b0 + gb) * 128, :].rearrange(
                    "(j n) d -> n j d", j=gb),
                in_=zt[:, :gb, :])

        for p in (psC, pC, psB, psAd, psDl, pBs, pB):
            p.release()
        cpool.release()

    import concourse.bacc as bacc_mod
    orig_tables = bacc_mod.get_activation_tables

    def _pinned(arch):
        t = orig_tables(arch)
        pin = "natural_log_exp_and_others"
        if pin in t:
            return {k: (v if k == pin else set()) for k, v in t.items()}
        return t

    bacc_mod.get_activation_tables = _pinned
    try:
        nc.compile()
    finally:
        bacc_mod.get_activation_tables = orig_tables
    return nc


def kernel(**inputs):
    import os
    from concourse.bass_utils import run_bass_kernel_spmd

    host = _build_host_data(inputs)
    nc = _build_program(host, phases=os.environ.get("GAT_PHASES", "ABC"))

    in_maps = []
    for c in range(NCORES):
        m = {
            "xT": host["xT"][c],
            "x_own": host["x_own"][c],
            "Wext": host["Wext"], "iotaP": host["iotaP"],
            "iotaRep": host["iotaRep"], "I128": host["I128"],
            "W1": host["W1"], "W2": host["W2"], "b1col": host["b1col"],
        }
        if not host["triv_bgat"]:
            m["bgat_r"] = np.tile(host["bias_gat"].reshape(1, -1), (128, 1))
        if not host["triv_bff2"]:
            m["bff2_r"] = np.tile(host["b_ff2"].reshape(1, -1), (128, 1))
        if not host["triv_gb1"]:
            m["g1_r"] = np.tile(host["gamma1"].reshape(1, -1), (128, 1))
            m["b1_r"] = np.tile(host["beta1"].reshape(1, -1), (128, 1))
        if not host["triv_gb2"]:
            m["g2_r"] = np.tile(host["gamma2"].reshape(1, -1), (128, 1))
            m["b2_r"] = np.tile(host["beta2"].reshape(1, -1), (128, 1))
        for s, sname in ((0, "lo"), (1, "hi")):
            sd = host["per_core"][c][s]
            m[f"gidx_{sname}"] = sd["gidx16"]
            m[f"dlt_{sname}"] = sd["dlt"]
        in_maps.append(m)

    trace = bool(int(os.environ.get("GAT_TRACE", "0")))
    res = run_bass_kernel_spmd(nc, in_maps, core_ids=list(range(NCORES)),
                               trace=trace)
    if trace and res.exec_time_ns:
        print(f"HW exec time: {res.exec_time_ns} ns")
    if bool(int(os.environ.get("GAT_TIME", "0"))):
        try:
            from concourse.timeline_sim import TimelineSim
            ts = TimelineSim(nc)
            dur = ts.simulate()
            print(f"HW exec time: {dur:.0f} ns (cost-model timeline estimate)")
        except Exception as e:
            print("timeline sim failed:", e)

    out = np.zeros((N, D), np.float32)
    for c in range(NCORES):
        lo_n = OWN * c
        hi_n = min(OWN * (c + 1), N)
        out[lo_n:hi_n] = res.results[c]["z"][: hi_n - lo_n]
    return out
